# revision 11
# baseline (speedup 1.0000x reference)
"""GCN + domain-adaptation CE loss on 8 Trainium2 NeuronCores.

Strategy (per spec sharding hint): nodes are sharded contiguously across the 8
cores (62500 each); edges are partitioned by destination shard so aggregation
is core-local.  Per core the device:
  phase 1: x' = rsqrt(deg) * (H_shard @ W)            (PE matmul, fp32)
  phase 2: AllGather of x' shards -> full gather table (collective)
  phase 3: per-edge gather of x'[src] from the table (indirect DMA, one
           instruction per multi-column chunk) followed by a dense per-node
           reduction.  Host pre-sorts edges by destination node (self loops
           included as ordinary edges) and pads each node's edge list to a
           degree bucket K, so the scatter-add becomes a dense strided
           tensor_reduce.  Node order is bucket-permuted; the loss is
           order-agnostic so y / deg are permuted identically on the host.
  phase 4: logits = dinv * agg + b; log-softmax cross-entropy; on-device
           reduction to a single partial sum per core.
Host work is layout only: sharding, sorting/bucketing edges, one-hot labels,
dtype casts, and summing the 8 scalar partials.
"""
import sys

sys.path.insert(0, "/opt/trn_rl_repo")

import numpy as np

N = 500000
C = 8
HID = 128
NCORES = 8
NSH = N // NCORES            # 62500 nodes per core
QP = 125                     # phase-1 uses 125 partitions x 500 nodes
QI = NSH // QP               # 500
ZROW = N                     # index of the all-zero row in the gather table
TBL_ROWS = N + 8
GMAX = 1024                  # gather cells per partition per instruction (B=8)


def _sigma(B):
    """SWDGE offset-consumption order for one [128, B] int32 offset block
    (HW-decoded): plain partition-fastest (slot q*B+m feeds cell m*128+q),
    except 15 'hole' cells at g=64j which re-consume another cell's slot.
    Hole cells are left uncovered by the reduce plan; since each hole's
    duplicated slot is also consumed later by its real cell, assigning
    st[:, sig] = want in g-order stores correct values everywhere."""
    assert B == 8
    sig = np.array([q * B + m for m in range(B) for q in range(128)], np.int64)
    for j in range(1, 15):
        q = 63 if j % 2 == 1 else 127
        sig[64 * j] = q * B + (j + 1) // 2
    sig[960] = 64 * B + 0
    return sig

# degree buckets: exact; max 63 so a node's edge run fits one 64-cell segment
K_LIST = list(range(1, 64))

_COMPILED = None  # (nc, layout)


def _plan_layout(deg_by_core):
    """Global bucket plan: same q_K per bucket on every core (shared NEFF)."""
    counts = np.zeros((NCORES, len(K_LIST)), np.int64)
    for r in range(NCORES):
        deg = deg_by_core[r]
        kidx = np.searchsorted(K_LIST, deg)
        assert kidx.max() < len(K_LIST), "degree exceeds largest bucket"
        counts[r] = np.bincount(kidx, minlength=len(K_LIST))
    mx = counts.max(axis=0)
    q = ((mx + 127) // 128).astype(np.int64)          # node-groups per bucket
    buckets = [(K_LIST[i], int(q[i])) for i in range(len(K_LIST)) if q[i] > 0]
    s_tot = sum(qk for _, qk in buckets)
    f_tot = sum(k * qk for k, qk in buckets)
    # chunk plan: uniform 1024-cell chunks of 16 segments; segment 0 holds
    # 64 usable cells, segments 1..15 hold 63 (cell 64j is a sigma hole,
    # gathers a duplicate and is not covered by any reduce piece)
    def segbase(j):
        return 64 * j + (0 if j == 0 else 1)

    def segcap(j):
        return 64 if j == 0 else 63

    chunks = []
    cur = []          # list of (K, n, col_off_in_chunk, slot_off)
    seg = 0
    s = 0             # cells used in current segment
    slot_off = 0
    for k, qk in buckets:
        left = qk
        while left > 0:
            room = (segcap(seg) - s) // k
            if room == 0:
                seg += 1
                s = 0
                if seg == 16:
                    chunks.append((cur, GMAX))
                    cur, seg = [], 0
                room = segcap(seg) // k
            n = min(left, room)
            cur.append((k, n, segbase(seg) + s, slot_off))
            s += n * k
            slot_off += n
            left -= n
    if cur:
        chunks.append((cur, GMAX))
    f_pad = sum(cc for _, cc in chunks)
    return buckets, chunks, s_tot, f_pad


def _build_program(layout):
    import concourse.bacc as bacc
    import concourse.bass as bass
    import concourse.mybir as mybir
    import concourse.tile as tile

    buckets, chunks, S, F = layout
    dt = mybir.dt
    nc = bacc.Bacc("TRN2", target_bir_lowering=False, debug=False,
                   num_devices=NCORES)

    ht = nc.dram_tensor("ht", [HID, NSH], dt.float32, kind="ExternalInput")
    wmat = nc.dram_tensor("wmat", [HID, C], dt.float32, kind="ExternalInput")
    degn = nc.dram_tensor("degn", [QP, QI], dt.float32, kind="ExternalInput")
    idxs = nc.dram_tensor("idxs", [128, F], dt.int32, kind="ExternalInput")
    degs = nc.dram_tensor("degs", [128, S], dt.float32, kind="ExternalInput")
    y1h = nc.dram_tensor("y1h", [128, S * C], dt.float32, kind="ExternalInput")
    brep = nc.dram_tensor("brep", [128, C], dt.float32, kind="ExternalInput")
    losss = nc.dram_tensor("losss", [1, 1], dt.float32, kind="ExternalOutput")

    xp = nc.dram_tensor("xp_shard", [NSH, C], dt.float32, kind="Internal")
    table = nc.dram_tensor("table", [TBL_ROWS, C], dt.float32,
                           kind="Internal", addr_space="Shared")

    with tile.TileContext(nc) as tc:
        # ---------------- phase 1: x' = rsqrt(deg) * (H @ W) ----------------
        with tc.tile_pool(name="p1", bufs=2) as p1, \
             tc.tile_pool(name="p1c", bufs=1) as p1c, \
             tc.tile_pool(name="ps1", bufs=4, space="PSUM") as ps1:
            wt = p1c.tile([HID, C], dt.float32)
            nc.sync.dma_start(out=wt[:], in_=wmat.ap())
            dg = p1c.tile([QP, QI], dt.float32)
            nc.sync.dma_start(out=dg[:], in_=degn.ap())
            sq = p1c.tile([QP, QI], dt.float32)
            nc.scalar.activation(out=sq[:], in_=dg[:],
                                 func=mybir.ActivationFunctionType.Sqrt)
            dinvn = p1c.tile([QP, QI], dt.float32)
            nc.vector.reciprocal(out=dinvn[:], in_=sq[:])

            stage = p1c.tile([QP, QI * C], dt.float32)
            CH = 25                       # matmul tiles per ht chunk
            for ch in range(QI // CH):
                htt = p1.tile([HID, CH * QP], dt.float32, tag="ht")
                nc.sync.dma_start(
                    out=htt[:], in_=ht.ap()[:, ch * CH * QP:(ch + 1) * CH * QP])
                for g in range(CH // 4 + (1 if CH % 4 else 0)):
                    ni = min(4, CH - g * 4)
                    pst = ps1.tile([QP, 4 * C], dt.float32, tag="mm",
                                   space="PSUM")
                    for j in range(ni):
                        i = g * 4 + j
                        nc.tensor.matmul(
                            out=pst[:, j * C:(j + 1) * C],
                            lhsT=htt[:, i * QP:(i + 1) * QP],
                            rhs=wt[:],
                            start=True, stop=True)
                    i0 = ch * CH + g * 4
                    nc.vector.tensor_copy(
                        out=stage[:, i0 * C:(i0 + ni) * C],
                        in_=pst[:, :ni * C])
            # scale rows by dinv (broadcast over channels)
            st3 = stage[:].rearrange("p (i c) -> p i c", c=C)
            nc.vector.tensor_tensor(
                out=st3, in0=st3,
                in1=dinvn[:].rearrange("p (i one) -> p i one", one=1).to_broadcast([QP, QI, C]),
                op=mybir.AluOpType.mult)
            nc.sync.dma_start(
                out=xp.ap().rearrange("(q i) c -> q (i c)", q=QP), in_=stage[:])
            # zero pad rows of the table
            zz = p1c.tile([TBL_ROWS - N, C], dt.float32)
            nc.gpsimd.memset(zz[:], 0.0)
            nc.sync.dma_start(out=table.ap()[N:TBL_ROWS, :], in_=zz[:])

        # ---------------- phase 2: AllGather x' shards ----------------------
        nc.gpsimd.collective_compute(
            "AllGather", bass.mybir.AluOpType.bypass,
            replica_groups=[list(range(NCORES))],
            ins=[xp.ap()], outs=[table.ap()[0:N, :]])

        # ---------------- phase 3: gather + dense bucket reduce -------------
        with tc.tile_pool(name="p3i", bufs=1) as p3i, \
             tc.tile_pool(name="p3g", bufs=3) as p3g, \
             tc.tile_pool(name="agg", bufs=1) as pagg:
            agg = pagg.tile([128, S * C], dt.float32)
            # dinv per slot (also used by phase 4)
            dgs = pagg.tile([128, S], dt.float32)
            nc.sync.dma_start(out=dgs[:], in_=degs.ap())
            sqs = pagg.tile([128, S], dt.float32)
            nc.scalar.activation(out=sqs[:], in_=dgs[:],
                                 func=mybir.ActivationFunctionType.Sqrt)
            dinv = pagg.tile([128, S], dt.float32)
            nc.vector.reciprocal(out=dinv[:], in_=sqs[:])
            col0 = 0
            for pieces, ccols in chunks:
                Bc = ccols // 128
                gb = p3g.tile([128, GMAX * C], dt.float32, tag="gb")
                base = gb[:]
                pstride = list(base.ap[0])[0]
                # one indirect DMA per PARTITION: a 3-level out AP based at
                # partition tp fires ccols descriptors (32B each) on that
                # partition; offsets come from a dedicated [128, Bc] block
                # (host stores them in SWDGE consumption order, see _sigma)
                for tp in range(128):
                    ich = p3i.tile([128, Bc], dt.int32, tag=f"ic{tp % 4}")
                    nc.sync.dma_start(
                        out=ich[:],
                        in_=idxs.ap()[:, col0 + tp * Bc:col0 + (tp + 1) * Bc])
                    app = bass.AP(base.tensor, base.offset + tp * pstride,
                                  [[pstride, 1], [C, ccols], [1, C]])
                    nc.gpsimd.indirect_dma_start(
                        out=app,
                        out_offset=None,
                        in_=table.ap(),
                        in_offset=bass.IndirectOffsetOnAxis(ap=ich[:], axis=0))
                for (k, n, coff, soff) in pieces:
                    base = gb[:]
                    src = bass.AP(
                        base.tensor,
                        base.offset + coff * C,
                        [list(base.ap[0]), [k * C, n], [1, C], [C, k]])
                    nc.vector.tensor_reduce(
                        out=agg[:, soff * C:(soff + n) * C].rearrange(
                            "p (i c) -> p i c", c=C),
                        in_=src, axis=mybir.AxisListType.X,
                        op=mybir.AluOpType.add)
                col0 += ccols

            # ---------------- phase 4: loss -----------------------------
            with tc.tile_pool(name="p4", bufs=1) as p4, \
                 tc.tile_pool(name="ps4", bufs=1, space="PSUM") as ps4:
                yt = p4.tile([128, S * C], dt.float32)
                nc.sync.dma_start(out=yt[:], in_=y1h.ap())
                bt = p4.tile([128, C], dt.float32)
                nc.sync.dma_start(out=bt[:], in_=brep.ap())

                a3 = agg[:].rearrange("p (i c) -> p i c", c=C)
                nc.vector.tensor_tensor(
                    out=a3, in0=a3,
                    in1=dinv[:].rearrange("p (i one) -> p i one", one=1).to_broadcast(
                        [128, S, C]),
                    op=mybir.AluOpType.mult)
                nc.vector.tensor_tensor(
                    out=a3, in0=a3,
                    in1=bt[:].rearrange("p (one c) -> p one c", one=1).to_broadcast(
                        [128, S, C]),
                    op=mybir.AluOpType.add)
                mx = p4.tile([128, S], dt.float32)
                nc.vector.tensor_reduce(out=mx[:], in_=a3,
                                        axis=mybir.AxisListType.X,
                                        op=mybir.AluOpType.max)
                ex = p4.tile([128, S * C], dt.float32)
                e3 = ex[:].rearrange("p (i c) -> p i c", c=C)
                nc.vector.tensor_tensor(
                    out=e3, in0=a3,
                    in1=mx[:].rearrange("p (i one) -> p i one", one=1).to_broadcast(
                        [128, S, C]),
                    op=mybir.AluOpType.subtract)
                nc.scalar.activation(out=ex[:], in_=ex[:],
                                     func=mybir.ActivationFunctionType.Exp)
                se = p4.tile([128, S], dt.float32)
                nc.vector.tensor_reduce(out=se[:], in_=e3,
                                        axis=mybir.AxisListType.X,
                                        op=mybir.AluOpType.add)
                nc.scalar.activation(out=se[:], in_=se[:],
                                     func=mybir.ActivationFunctionType.Ln)
                lse = p4.tile([128, S], dt.float32)
                nc.vector.tensor_tensor(out=lse[:], in0=se[:], in1=mx[:],
                                        op=mybir.AluOpType.add)
                # mask = rowsum(y1h) computed BEFORE y3 is overwritten
                y3 = yt[:].rearrange("p (i c) -> p i c", c=C)
                msk = p4.tile([128, S], dt.float32)
                nc.vector.tensor_reduce(out=msk[:], in_=y3,
                                        axis=mybir.AxisListType.X,
                                        op=mybir.AluOpType.add)
                nc.vector.tensor_tensor(out=y3, in0=y3, in1=a3,
                                        op=mybir.AluOpType.mult)
                oy = p4.tile([128, S], dt.float32)
                nc.vector.tensor_reduce(out=oy[:], in_=y3,
                                        axis=mybir.AxisListType.X,
                                        op=mybir.AluOpType.add)
                term = p4.tile([128, S], dt.float32)
                nc.vector.tensor_tensor(out=term[:], in0=msk[:], in1=lse[:],
                                        op=mybir.AluOpType.mult)
                nc.vector.tensor_tensor(out=term[:], in0=term[:], in1=oy[:],
                                        op=mybir.AluOpType.subtract)
                part = p4.tile([128, 1], dt.float32)
                nc.vector.tensor_reduce(out=part[:], in_=term[:],
                                        axis=mybir.AxisListType.X,
                                        op=mybir.AluOpType.add)
                ones = p4.tile([128, 1], dt.float32)
                nc.gpsimd.memset(ones[:], 1.0)
                tot = ps4.tile([1, 1], dt.float32, space="PSUM")
                nc.tensor.matmul(out=tot[:], lhsT=part[:], rhs=ones[:],
                                 start=True, stop=True)
                res = p4.tile([1, 1], dt.float32)
                nc.vector.tensor_copy(out=res[:], in_=tot[:])
                nc.sync.dma_start(out=losss.ap(), in_=res[:])

    nc.compile()
    return nc


def _prep_inputs(hiddens, edge_index, y, W, b):
    src = np.asarray(edge_index[0], dtype=np.int64)
    dst = np.asarray(edge_index[1], dtype=np.int64)
    hid = np.asarray(hiddens, dtype=np.float32)
    yv = np.asarray(y, dtype=np.int64)

    shard = dst // NSH
    order = np.argsort(shard, kind="stable")
    src_s, dst_s = src[order], dst[order]
    bounds = np.searchsorted(shard[order], np.arange(NCORES + 1))

    deg_by_core, csr = [], []
    for r in range(NCORES):
        s, e = bounds[r], bounds[r + 1]
        dl = dst_s[s:e] - r * NSH
        sg = src_s[s:e]
        o2 = np.argsort(dl, kind="stable")
        dl, sg = dl[o2], sg[o2]
        deg = np.bincount(dl, minlength=NSH) + 1   # incl self loop
        csr.append((np.cumsum(np.bincount(dl, minlength=NSH)), sg))
        deg_by_core.append(deg)

    layout = _plan_layout(deg_by_core)
    buckets, chunks, S, F = layout

    in_maps = []
    for r in range(NCORES):
        deg = deg_by_core[r]
        endptr, sg = csr[r]
        startptr = np.concatenate([[0], endptr[:-1]])
        kidx = np.searchsorted(K_LIST, deg)
        assert deg.max() <= K_LIST[-1], "degree exceeds largest bucket"

        idx_arr = np.full((128, F), ZROW, np.int32)
        degs_arr = np.ones((128, S), np.float32)
        y1h_arr = np.zeros((128, S, C), np.float32)

        # column base per node-group, following the (chunk-padded) piece plan
        gcols = {k: [] for k, _ in buckets}
        cbase = 0
        for pieces, ccols in chunks:
            for (k, pn, coff, soff) in pieces:
                for t in range(pn):
                    gcols[k].append(cbase + coff + t * k)
            cbase += ccols

        slot0 = 0
        for bi, (k, qk) in enumerate(buckets):
            ki = K_LIST.index(k)
            nodes = np.where(kidx == ki)[0]
            for j, n in enumerate(nodes):
                p, i = j % 128, j // 128
                c0 = gcols[k][i]
                nsrc = sg[startptr[n]:endptr[n]]
                idx_arr[p, c0:c0 + len(nsrc)] = nsrc
                idx_arr[p, c0 + len(nsrc)] = r * NSH + n   # self loop
                degs_arr[p, slot0 + i] = deg[n]
                y1h_arr[p, slot0 + i, yv[r * NSH + n]] = 1.0
            slot0 += qk

        # re-store indices in SWDGE consumption order: per chunk, per target
        # partition p, a [128, Bc] block at DRAM cols [c0 + p*Bc, +Bc)
        idx_store = np.empty_like(idx_arr)
        c0 = 0
        for pieces, ccols in chunks:
            Bc = ccols // 128
            sig = _sigma(Bc)
            want = idx_arr[:, c0:c0 + ccols]
            st = np.full_like(want, ZROW)
            st[:, sig] = want
            idx_store[:, c0:c0 + ccols] = st.reshape(
                128, 128, Bc).transpose(1, 0, 2).reshape(128, 128 * Bc)
            c0 += ccols
        idx_arr = idx_store

        hs = hid[r * NSH:(r + 1) * NSH]                     # [NSH, 128]
        # column order (i*QP + q) -> node q*QI + i
        ht = np.ascontiguousarray(
            hs.T.reshape(HID, QP, QI).transpose(0, 2, 1).reshape(HID, NSH))
        degn = np.ascontiguousarray(
            deg.astype(np.float32).reshape(QP, QI))

        in_maps.append({
            "ht": ht,
            "wmat": np.asarray(W, np.float32),
            "degn": degn,
            "idxs": idx_arr,
            "degs": degs_arr,
            "y1h": y1h_arr.reshape(128, S * C),
            "brep": np.broadcast_to(np.asarray(b, np.float32),
                                    (128, C)).copy(),
        })
    return in_maps, layout


def kernel(hiddens, edge_index, y, q_edge_index, W, b, trace=False):
    global _COMPILED
    from concourse import bass_utils

    in_maps, layout = _prep_inputs(hiddens, edge_index, y, W, b)
    if _COMPILED is None:
        _COMPILED = _build_program(layout)
    nc = _COMPILED
    res = bass_utils.run_bass_kernel_spmd(
        nc, in_maps, core_ids=list(range(NCORES)), trace=trace)
    total = sum(float(r["losss"][0, 0]) for r in res.results)
    out = np.float32(total / N)
    if trace:
        return out, res
    return out


# revision 14
# speedup vs baseline: 7.3403x; 7.3403x over previous
"""GCN + domain-adaptation CE loss on 8 Trainium2 NeuronCores.

Strategy (per spec sharding hint): nodes are sharded contiguously across the 8
cores (62500 each); edges are partitioned by destination shard so aggregation
is core-local.  Per core the device:
  phase 1: x' = rsqrt(deg) * (H_shard @ W)            (PE matmul, fp32)
  phase 2: AllGather of x' shards -> full gather table (collective)
  phase 3: per-edge gather of x'[src] from the table (indirect DMA) followed
           by a dense per-node reduction.  Host pre-sorts edges by destination
           node and pads each node's edge list to a degree bucket K, so the
           scatter-add becomes a dense strided tensor_reduce.  Node order is
           bucket-permuted; the loss is order-agnostic so y / deg are permuted
           identically on the host.
  phase 4: logits = dinv * agg + b; log-softmax cross-entropy; on-device
           reduction to a single partial sum per core.
Host work is layout only: sharding, sorting/bucketing edges, one-hot labels,
dtype casts, and summing the 8 scalar partials.
"""
import sys

sys.path.insert(0, "/opt/trn_rl_repo")

import numpy as np

N = 500000
C = 8
HID = 128
NCORES = 8
NSH = N // NCORES            # 62500 nodes per core
QP = 125                     # phase-1 uses 125 partitions x 500 nodes
QI = NSH // QP               # 500
ZROW = N                     # index of the all-zero row in the gather table
TBL_ROWS = N + 8
GMAX = 384                   # max gather columns per chunk (SBUF budget)

# degree buckets: DP-optimal boundaries for the seed-0 degree histogram
# (minimizes sum k*ceil(max-core-count/128) = gather instructions), safety tail
K_LIST = [3, 8, 9, 10, 11, 12, 13, 14, 15, 16, 17, 18, 19, 20, 22, 23, 24,
          25, 29, 38, 80, 128, 256, 512]

_COMPILED = None  # (nc, layout)


def _plan_layout(deg_by_core):
    """Global bucket plan: same q_K per bucket on every core (shared NEFF)."""
    counts = np.zeros((NCORES, len(K_LIST)), np.int64)
    for r in range(NCORES):
        deg = deg_by_core[r]
        kidx = np.searchsorted(K_LIST, np.maximum(deg - 1, 1))
        assert kidx.max() < len(K_LIST), "degree exceeds largest bucket"
        counts[r] = np.bincount(kidx, minlength=len(K_LIST))
    mx = counts.max(axis=0)
    q = ((mx + 127) // 128).astype(np.int64)          # node-groups per bucket
    buckets = [(K_LIST[i], int(q[i])) for i in range(len(K_LIST)) if q[i] > 0]
    s_tot = sum(qk for _, qk in buckets)
    f_tot = sum(k * qk for k, qk in buckets)
    # chunk plan: pieces (K, node_count, col_off, slot_off) split at <=GMAX cols
    chunks = []
    cur = []          # list of (K, n, col_off_in_chunk, slot_off)
    cur_cols = 0
    slot_off = 0
    for k, qk in buckets:
        left = qk
        while left > 0:
            room = (GMAX - cur_cols) // k
            if room == 0:
                chunks.append((cur, cur_cols))
                cur, cur_cols = [], 0
                room = GMAX // k
            n = min(left, room)
            cur.append((k, n, cur_cols, slot_off))
            cur_cols += n * k
            slot_off += n
            left -= n
    if cur:
        chunks.append((cur, cur_cols))
    return buckets, chunks, s_tot, f_tot


def _build_program(layout):
    import concourse.bacc as bacc
    import concourse.bass as bass
    import concourse.mybir as mybir
    import concourse.tile as tile

    buckets, chunks, S, F = layout
    dt = mybir.dt
    nc = bacc.Bacc("TRN2", target_bir_lowering=False, debug=False,
                   num_devices=NCORES)

    ht = nc.dram_tensor("ht", [HID, NSH], dt.float32, kind="ExternalInput")
    ht2 = nc.dram_tensor("ht2", [HID, S * 128], dt.float32, kind="ExternalInput")
    wmat = nc.dram_tensor("wmat", [HID, C], dt.float32, kind="ExternalInput")
    degn = nc.dram_tensor("degn", [QP, QI], dt.float32, kind="ExternalInput")
    idxs = nc.dram_tensor("idxs", [128, F], dt.int32, kind="ExternalInput")
    degs = nc.dram_tensor("degs", [128, S], dt.float32, kind="ExternalInput")
    y1h = nc.dram_tensor("y1h", [128, S * C], dt.float32, kind="ExternalInput")
    brep = nc.dram_tensor("brep", [128, C], dt.float32, kind="ExternalInput")
    losss = nc.dram_tensor("losss", [1, 1], dt.float32, kind="ExternalOutput")

    xp = nc.dram_tensor("xp_shard", [NSH, C], dt.float32, kind="Internal")
    table = nc.dram_tensor("table", [TBL_ROWS, C], dt.float32,
                           kind="Internal", addr_space="Shared")

    with tile.TileContext(nc) as tc:
        # ---------------- phase 1: x' = rsqrt(deg) * (H @ W) ----------------
        with tc.tile_pool(name="p1", bufs=2) as p1, \
             tc.tile_pool(name="p1c", bufs=1) as p1c, \
             tc.tile_pool(name="ps1", bufs=4, space="PSUM") as ps1:
            wt = p1c.tile([HID, C], dt.float32)
            nc.sync.dma_start(out=wt[:], in_=wmat.ap())
            dg = p1c.tile([QP, QI], dt.float32)
            nc.sync.dma_start(out=dg[:], in_=degn.ap())
            sq = p1c.tile([QP, QI], dt.float32)
            nc.scalar.activation(out=sq[:], in_=dg[:],
                                 func=mybir.ActivationFunctionType.Sqrt)
            dinvn = p1c.tile([QP, QI], dt.float32)
            nc.vector.reciprocal(out=dinvn[:], in_=sq[:])

            stage = p1c.tile([QP, QI * C], dt.float32)
            CH = 25                       # matmul tiles per ht chunk
            for ch in range(QI // CH):
                htt = p1.tile([HID, CH * QP], dt.float32, tag="ht")
                nc.sync.dma_start(
                    out=htt[:], in_=ht.ap()[:, ch * CH * QP:(ch + 1) * CH * QP])
                for g in range(CH // 4 + (1 if CH % 4 else 0)):
                    ni = min(4, CH - g * 4)
                    pst = ps1.tile([QP, 4 * C], dt.float32, tag="mm",
                                   space="PSUM")
                    for j in range(ni):
                        i = g * 4 + j
                        nc.tensor.matmul(
                            out=pst[:, j * C:(j + 1) * C],
                            lhsT=htt[:, i * QP:(i + 1) * QP],
                            rhs=wt[:],
                            start=True, stop=True)
                    i0 = ch * CH + g * 4
                    nc.vector.tensor_copy(
                        out=stage[:, i0 * C:(i0 + ni) * C],
                        in_=pst[:, :ni * C])
            # scale rows by dinv (broadcast over channels)
            st3 = stage[:].rearrange("p (i c) -> p i c", c=C)
            nc.vector.tensor_tensor(
                out=st3, in0=st3,
                in1=dinvn[:].rearrange("p (i one) -> p i one", one=1).to_broadcast([QP, QI, C]),
                op=mybir.AluOpType.mult)
            nc.sync.dma_start(
                out=xp.ap().rearrange("(q i) c -> q (i c)", q=QP), in_=stage[:])
            # zero pad rows of the table
            zz = p1c.tile([TBL_ROWS - N, C], dt.float32)
            nc.gpsimd.memset(zz[:], 0.0)
            nc.sync.dma_start(out=table.ap()[N:TBL_ROWS, :], in_=zz[:])

        # ---------------- phase 2: AllGather x' shards ----------------------
        nc.gpsimd.collective_compute(
            "AllGather", bass.mybir.AluOpType.bypass,
            replica_groups=[list(range(NCORES))],
            ins=[xp.ap()], outs=[table.ap()[0:N, :]])

        # ---------------- phase 3: gather + dense bucket reduce -------------
        with tc.tile_pool(name="p3i", bufs=1) as p3i, \
             tc.tile_pool(name="p3g", bufs=3) as p3g, \
             tc.tile_pool(name="agg", bufs=1) as pagg, \
             tc.tile_pool(name="p1b", bufs=2) as p1b, \
             tc.tile_pool(name="ps3", bufs=4, space="PSUM") as ps3:
            agg = pagg.tile([128, S * C], dt.float32)
            # dinv per slot (also used by phase 4)
            dgs = pagg.tile([128, S], dt.float32)
            nc.sync.dma_start(out=dgs[:], in_=degs.ap())
            sqs = pagg.tile([128, S], dt.float32)
            nc.scalar.activation(out=sqs[:], in_=dgs[:],
                                 func=mybir.ActivationFunctionType.Sqrt)
            dinv = pagg.tile([128, S], dt.float32)
            nc.vector.reciprocal(out=dinv[:], in_=sqs[:])
            # phase 1b: self-loop term x'_n = dinv_n * (H_n @ W) directly in
            # slot order (no gather needed for self loops)
            agginit = pagg.tile([128, S * C], dt.float32)
            wt2 = pagg.tile([HID, C], dt.float32)
            nc.sync.dma_start(out=wt2[:], in_=wmat.ap())
            B2 = 16
            nchk = (S + B2 - 1) // B2
            for ch in range(nchk):
                i0 = ch * B2
                nb = min(B2, S - i0)
                h2 = p1b.tile([HID, B2 * 128], dt.float32, tag="h2")
                nc.sync.dma_start(
                    out=h2[:, :nb * 128],
                    in_=ht2.ap()[:, i0 * 128:(i0 + nb) * 128])
                for j in range(nb):
                    i = i0 + j
                    p2 = ps3.tile([128, C], dt.float32, tag="mm2",
                                  space="PSUM")
                    nc.tensor.matmul(out=p2[:],
                                     lhsT=h2[:, j * 128:(j + 1) * 128],
                                     rhs=wt2[:], start=True, stop=True)
                    nc.vector.tensor_scalar_mul(
                        out=agginit[:, i * C:(i + 1) * C], in0=p2[:],
                        scalar1=dinv[:, i:i + 1])
            itall = p3i.tile([128, F], dt.int32)
            nc.sync.dma_start(out=itall[:], in_=idxs.ap())
            col0 = 0
            for pieces, ccols in chunks:
                gb = p3g.tile([128, GMAX * C], dt.float32, tag="gb")
                for g in range(ccols):
                    nc.gpsimd.indirect_dma_start(
                        out=gb[:, g * C:(g + 1) * C],
                        out_offset=None,
                        in_=table.ap(),
                        in_offset=bass.IndirectOffsetOnAxis(
                            ap=itall[:, col0 + g:col0 + g + 1], axis=0))
                for (k, n, coff, soff) in pieces:
                    base = gb[:]
                    src = bass.AP(
                        base.tensor,
                        base.offset + coff * C,
                        [list(base.ap[0]), [k * C, n], [1, C], [C, k]])
                    nc.vector.tensor_reduce(
                        out=agg[:, soff * C:(soff + n) * C].rearrange(
                            "p (i c) -> p i c", c=C),
                        in_=src, axis=mybir.AxisListType.X,
                        op=mybir.AluOpType.add)
                col0 += ccols

            nc.vector.tensor_tensor(out=agg[:], in0=agg[:], in1=agginit[:],
                                    op=mybir.AluOpType.add)

            # ---------------- phase 4: loss -----------------------------
            with tc.tile_pool(name="p4", bufs=1) as p4, \
                 tc.tile_pool(name="ps4", bufs=1, space="PSUM") as ps4:
                yt = p4.tile([128, S * C], dt.float32)
                nc.sync.dma_start(out=yt[:], in_=y1h.ap())
                bt = p4.tile([128, C], dt.float32)
                nc.sync.dma_start(out=bt[:], in_=brep.ap())

                a3 = agg[:].rearrange("p (i c) -> p i c", c=C)
                nc.vector.tensor_tensor(
                    out=a3, in0=a3,
                    in1=dinv[:].rearrange("p (i one) -> p i one", one=1).to_broadcast(
                        [128, S, C]),
                    op=mybir.AluOpType.mult)
                nc.vector.tensor_tensor(
                    out=a3, in0=a3,
                    in1=bt[:].rearrange("p (one c) -> p one c", one=1).to_broadcast(
                        [128, S, C]),
                    op=mybir.AluOpType.add)
                mx = p4.tile([128, S], dt.float32)
                nc.vector.tensor_reduce(out=mx[:], in_=a3,
                                        axis=mybir.AxisListType.X,
                                        op=mybir.AluOpType.max)
                ex = p4.tile([128, S * C], dt.float32)
                e3 = ex[:].rearrange("p (i c) -> p i c", c=C)
                nc.vector.tensor_tensor(
                    out=e3, in0=a3,
                    in1=mx[:].rearrange("p (i one) -> p i one", one=1).to_broadcast(
                        [128, S, C]),
                    op=mybir.AluOpType.subtract)
                nc.scalar.activation(out=ex[:], in_=ex[:],
                                     func=mybir.ActivationFunctionType.Exp)
                se = p4.tile([128, S], dt.float32)
                nc.vector.tensor_reduce(out=se[:], in_=e3,
                                        axis=mybir.AxisListType.X,
                                        op=mybir.AluOpType.add)
                nc.scalar.activation(out=se[:], in_=se[:],
                                     func=mybir.ActivationFunctionType.Ln)
                lse = p4.tile([128, S], dt.float32)
                nc.vector.tensor_tensor(out=lse[:], in0=se[:], in1=mx[:],
                                        op=mybir.AluOpType.add)
                # mask = rowsum(y1h) computed BEFORE y3 is overwritten
                y3 = yt[:].rearrange("p (i c) -> p i c", c=C)
                msk = p4.tile([128, S], dt.float32)
                nc.vector.tensor_reduce(out=msk[:], in_=y3,
                                        axis=mybir.AxisListType.X,
                                        op=mybir.AluOpType.add)
                nc.vector.tensor_tensor(out=y3, in0=y3, in1=a3,
                                        op=mybir.AluOpType.mult)
                oy = p4.tile([128, S], dt.float32)
                nc.vector.tensor_reduce(out=oy[:], in_=y3,
                                        axis=mybir.AxisListType.X,
                                        op=mybir.AluOpType.add)
                term = p4.tile([128, S], dt.float32)
                nc.vector.tensor_tensor(out=term[:], in0=msk[:], in1=lse[:],
                                        op=mybir.AluOpType.mult)
                nc.vector.tensor_tensor(out=term[:], in0=term[:], in1=oy[:],
                                        op=mybir.AluOpType.subtract)
                part = p4.tile([128, 1], dt.float32)
                nc.vector.tensor_reduce(out=part[:], in_=term[:],
                                        axis=mybir.AxisListType.X,
                                        op=mybir.AluOpType.add)
                ones = p4.tile([128, 1], dt.float32)
                nc.gpsimd.memset(ones[:], 1.0)
                tot = ps4.tile([1, 1], dt.float32, space="PSUM")
                nc.tensor.matmul(out=tot[:], lhsT=part[:], rhs=ones[:],
                                 start=True, stop=True)
                res = p4.tile([1, 1], dt.float32)
                nc.vector.tensor_copy(out=res[:], in_=tot[:])
                nc.sync.dma_start(out=losss.ap(), in_=res[:])

    nc.compile()
    return nc


def _prep_inputs(hiddens, edge_index, y, W, b):
    src = np.asarray(edge_index[0], dtype=np.int64)
    dst = np.asarray(edge_index[1], dtype=np.int64)
    hid = np.asarray(hiddens, dtype=np.float32)
    yv = np.asarray(y, dtype=np.int64)

    shard = dst // NSH
    order = np.argsort(shard, kind="stable")
    src_s, dst_s = src[order], dst[order]
    bounds = np.searchsorted(shard[order], np.arange(NCORES + 1))

    deg_by_core, csr = [], []
    for r in range(NCORES):
        s, e = bounds[r], bounds[r + 1]
        dl = dst_s[s:e] - r * NSH
        sg = src_s[s:e]
        o2 = np.argsort(dl, kind="stable")
        dl, sg = dl[o2], sg[o2]
        deg = np.bincount(dl, minlength=NSH) + 1   # incl self loop
        csr.append((np.cumsum(np.bincount(dl, minlength=NSH)), sg))
        deg_by_core.append(deg)

    layout = _plan_layout(deg_by_core)
    buckets, chunks, S, F = layout

    in_maps = []
    for r in range(NCORES):
        deg = deg_by_core[r]
        endptr, sg = csr[r]
        startptr = np.concatenate([[0], endptr[:-1]])
        kidx = np.searchsorted(K_LIST, np.maximum(deg - 1, 1))

        idx_arr = np.full((128, F), ZROW, np.int32)
        degs_arr = np.ones((128, S), np.float32)
        y1h_arr = np.zeros((128, S, C), np.float32)
        slot_node = np.full(S * 128, -1, np.int64)   # (i*128+p) -> node

        col0 = 0
        slot0 = 0
        for bi, (k, qk) in enumerate(buckets):
            ki = K_LIST.index(k)
            nodes = np.where(kidx == ki)[0]
            for j, n in enumerate(nodes):
                p, i = j % 128, j // 128
                c0 = col0 + i * k
                nsrc = sg[startptr[n]:endptr[n]]
                idx_arr[p, c0:c0 + len(nsrc)] = nsrc
                degs_arr[p, slot0 + i] = deg[n]
                y1h_arr[p, slot0 + i, yv[r * NSH + n]] = 1.0
                slot_node[(slot0 + i) * 128 + p] = n
            col0 += qk * k
            slot0 += qk

        hs = hid[r * NSH:(r + 1) * NSH]                     # [NSH, 128]
        # column order (i*QP + q) -> node q*QI + i
        ht = np.ascontiguousarray(
            hs.T.reshape(HID, QP, QI).transpose(0, 2, 1).reshape(HID, NSH))
        degn = np.ascontiguousarray(
            deg.astype(np.float32).reshape(QP, QI))

        ht2 = np.zeros((HID, S * 128), np.float32)
        valid = slot_node >= 0
        ht2[:, np.where(valid)[0]] = hs.T[:, slot_node[valid]]

        in_maps.append({
            "ht": ht,
            "ht2": ht2,
            "wmat": np.asarray(W, np.float32),
            "degn": degn,
            "idxs": idx_arr,
            "degs": degs_arr,
            "y1h": y1h_arr.reshape(128, S * C),
            "brep": np.broadcast_to(np.asarray(b, np.float32),
                                    (128, C)).copy(),
        })
    return in_maps, layout


def kernel(hiddens, edge_index, y, q_edge_index, W, b, trace=False):
    global _COMPILED
    from concourse import bass_utils

    in_maps, layout = _prep_inputs(hiddens, edge_index, y, W, b)
    if _COMPILED is None:
        _COMPILED = _build_program(layout)
    nc = _COMPILED
    res = bass_utils.run_bass_kernel_spmd(
        nc, in_maps, core_ids=list(range(NCORES)), trace=trace)
    total = sum(float(r["losss"][0, 0]) for r in res.results)
    out = np.float32(total / N)
    if trace:
        return out, res
    return out



# revision 15
# speedup vs baseline: 7.4725x; 1.0180x over previous
"""GCN + domain-adaptation CE loss on 8 Trainium2 NeuronCores.

Strategy (per spec sharding hint): nodes are sharded contiguously across the 8
cores (62500 each); edges are partitioned by destination shard so aggregation
is core-local.  Per core the device:
  phase 1: x' = rsqrt(deg) * (H_shard @ W)            (PE matmul, fp32)
  phase 2: AllGather of x' shards -> full gather table (collective)
  phase 3: per-edge gather of x'[src] from the table (indirect DMA) followed
           by a dense per-node reduction.  Host pre-sorts edges by destination
           node and pads each node's edge list to a degree bucket K, so the
           scatter-add becomes a dense strided tensor_reduce.  Node order is
           bucket-permuted; the loss is order-agnostic so y / deg are permuted
           identically on the host.
  phase 4: logits = dinv * agg + b; log-softmax cross-entropy; on-device
           reduction to a single partial sum per core.
Host work is layout only: sharding, sorting/bucketing edges, one-hot labels,
dtype casts, and summing the 8 scalar partials.
"""
import sys

sys.path.insert(0, "/opt/trn_rl_repo")

import numpy as np

N = 500000
C = 8
HID = 128
NCORES = 8
NSH = N // NCORES            # 62500 nodes per core
QP = 125                     # phase-1 uses 125 partitions x 500 nodes
QI = NSH // QP               # 500
ZROW = N                     # index of the all-zero row in the gather table
TBL_ROWS = N + 8
GMAX = 384                   # max gather columns per chunk (SBUF budget)

# degree buckets: DP-optimal boundaries for the seed-0 degree histogram
# (minimizes sum k*ceil(max-core-count/128) = gather instructions), safety tail
K_LIST = [6, 7, 8, 9, 10, 11, 12, 13, 14, 15, 16, 17, 18, 19, 20, 21, 23,
          24, 25, 29, 38, 64, 128, 256, 512]

_COMPILED = None  # (nc, layout)


def _plan_layout(deg_by_core):
    """Global bucket plan: same q_K per bucket on every core (shared NEFF)."""
    counts = np.zeros((NCORES, len(K_LIST)), np.int64)
    for r in range(NCORES):
        deg = deg_by_core[r]
        kidx = np.searchsorted(K_LIST, np.maximum(deg - 1, 1))
        assert kidx.max() < len(K_LIST), "degree exceeds largest bucket"
        counts[r] = np.bincount(kidx, minlength=len(K_LIST))
    mx = counts.max(axis=0)
    q = ((mx + 127) // 128).astype(np.int64)          # node-groups per bucket
    buckets = [(K_LIST[i], int(q[i])) for i in range(len(K_LIST)) if q[i] > 0]
    s_tot = sum(qk for _, qk in buckets)
    f_tot = sum(k * qk for k, qk in buckets)
    # chunk plan: pieces (K, node_count, col_off, slot_off) split at <=GMAX cols
    chunks = []
    cur = []          # list of (K, n, col_off_in_chunk, slot_off)
    cur_cols = 0
    slot_off = 0
    for k, qk in buckets:
        left = qk
        while left > 0:
            room = (GMAX - cur_cols) // k
            if room == 0:
                chunks.append((cur, cur_cols))
                cur, cur_cols = [], 0
                room = GMAX // k
            n = min(left, room)
            cur.append((k, n, cur_cols, slot_off))
            cur_cols += n * k
            slot_off += n
            left -= n
    if cur:
        chunks.append((cur, cur_cols))
    return buckets, chunks, s_tot, f_tot


def _build_program(layout):
    import concourse.bacc as bacc
    import concourse.bass as bass
    import concourse.mybir as mybir
    import concourse.tile as tile

    buckets, chunks, S, F = layout
    dt = mybir.dt
    nc = bacc.Bacc("TRN2", target_bir_lowering=False, debug=False,
                   num_devices=NCORES)

    ht = nc.dram_tensor("ht", [HID, NSH], dt.float32, kind="ExternalInput")
    ht2 = nc.dram_tensor("ht2", [HID, S * 128], dt.float32, kind="ExternalInput")
    wmat = nc.dram_tensor("wmat", [HID, C], dt.float32, kind="ExternalInput")
    degn = nc.dram_tensor("degn", [QP, QI], dt.float32, kind="ExternalInput")
    idxs = nc.dram_tensor("idxs", [128, F], dt.int32, kind="ExternalInput")
    degs = nc.dram_tensor("degs", [128, S], dt.float32, kind="ExternalInput")
    y1h = nc.dram_tensor("y1h", [128, S * C], dt.float32, kind="ExternalInput")
    brep = nc.dram_tensor("brep", [128, C], dt.float32, kind="ExternalInput")
    losss = nc.dram_tensor("losss", [1, 1], dt.float32, kind="ExternalOutput")

    xp = nc.dram_tensor("xp_shard", [NSH, C], dt.float32, kind="Internal")
    table = nc.dram_tensor("table", [TBL_ROWS, C], dt.float32,
                           kind="Internal", addr_space="Shared")

    with tile.TileContext(nc) as tc:
        # ---------------- phase 1: x' = rsqrt(deg) * (H @ W) ----------------
        with tc.tile_pool(name="p1", bufs=2) as p1, \
             tc.tile_pool(name="p1c", bufs=1) as p1c, \
             tc.tile_pool(name="ps1", bufs=4, space="PSUM") as ps1:
            wt = p1c.tile([HID, C], dt.float32)
            nc.sync.dma_start(out=wt[:], in_=wmat.ap())
            dg = p1c.tile([QP, QI], dt.float32)
            nc.sync.dma_start(out=dg[:], in_=degn.ap())
            sq = p1c.tile([QP, QI], dt.float32)
            nc.scalar.activation(out=sq[:], in_=dg[:],
                                 func=mybir.ActivationFunctionType.Sqrt)
            dinvn = p1c.tile([QP, QI], dt.float32)
            nc.vector.reciprocal(out=dinvn[:], in_=sq[:])

            stage = p1c.tile([QP, QI * C], dt.float32)
            CH = 25                       # matmul tiles per ht chunk
            for ch in range(QI // CH):
                htt = p1.tile([HID, CH * QP], dt.float32, tag="ht")
                nc.sync.dma_start(
                    out=htt[:], in_=ht.ap()[:, ch * CH * QP:(ch + 1) * CH * QP])
                for g in range(CH // 4 + (1 if CH % 4 else 0)):
                    ni = min(4, CH - g * 4)
                    pst = ps1.tile([QP, 4 * C], dt.float32, tag="mm",
                                   space="PSUM")
                    for j in range(ni):
                        i = g * 4 + j
                        nc.tensor.matmul(
                            out=pst[:, j * C:(j + 1) * C],
                            lhsT=htt[:, i * QP:(i + 1) * QP],
                            rhs=wt[:],
                            start=True, stop=True)
                    i0 = ch * CH + g * 4
                    nc.vector.tensor_copy(
                        out=stage[:, i0 * C:(i0 + ni) * C],
                        in_=pst[:, :ni * C])
            # scale rows by dinv (broadcast over channels)
            st3 = stage[:].rearrange("p (i c) -> p i c", c=C)
            nc.vector.tensor_tensor(
                out=st3, in0=st3,
                in1=dinvn[:].rearrange("p (i one) -> p i one", one=1).to_broadcast([QP, QI, C]),
                op=mybir.AluOpType.mult)
            nc.sync.dma_start(
                out=xp.ap().rearrange("(q i) c -> q (i c)", q=QP), in_=stage[:])
            # zero pad rows of the table
            zz = p1c.tile([TBL_ROWS - N, C], dt.float32)
            nc.gpsimd.memset(zz[:], 0.0)
            nc.sync.dma_start(out=table.ap()[N:TBL_ROWS, :], in_=zz[:])

        # ---------------- phase 2: AllGather x' shards ----------------------
        nc.gpsimd.collective_compute(
            "AllGather", bass.mybir.AluOpType.bypass,
            replica_groups=[list(range(NCORES))],
            ins=[xp.ap()], outs=[table.ap()[0:N, :]])

        # ---------------- phase 3: gather + dense bucket reduce -------------
        with tc.tile_pool(name="p3i", bufs=1) as p3i, \
             tc.tile_pool(name="p3g", bufs=3) as p3g, \
             tc.tile_pool(name="agg", bufs=1) as pagg, \
             tc.tile_pool(name="p1b", bufs=2) as p1b, \
             tc.tile_pool(name="ps3", bufs=4, space="PSUM") as ps3:
            agg = pagg.tile([128, S * C], dt.float32)
            # dinv per slot (also used by phase 4)
            dgs = pagg.tile([128, S], dt.float32)
            nc.sync.dma_start(out=dgs[:], in_=degs.ap())
            sqs = pagg.tile([128, S], dt.float32)
            nc.scalar.activation(out=sqs[:], in_=dgs[:],
                                 func=mybir.ActivationFunctionType.Sqrt)
            dinv = pagg.tile([128, S], dt.float32)
            nc.vector.reciprocal(out=dinv[:], in_=sqs[:])
            # phase 1b: self-loop term x'_n = dinv_n * (H_n @ W) directly in
            # slot order (no gather needed for self loops)
            agginit = pagg.tile([128, S * C], dt.float32)
            wt2 = pagg.tile([HID, C], dt.float32)
            nc.sync.dma_start(out=wt2[:], in_=wmat.ap())
            B2 = 16
            nchk = (S + B2 - 1) // B2
            for ch in range(nchk):
                i0 = ch * B2
                nb = min(B2, S - i0)
                h2 = p1b.tile([HID, B2 * 128], dt.float32, tag="h2")
                nc.sync.dma_start(
                    out=h2[:, :nb * 128],
                    in_=ht2.ap()[:, i0 * 128:(i0 + nb) * 128])
                for j in range(nb):
                    i = i0 + j
                    p2 = ps3.tile([128, C], dt.float32, tag="mm2",
                                  space="PSUM")
                    nc.tensor.matmul(out=p2[:],
                                     lhsT=h2[:, j * 128:(j + 1) * 128],
                                     rhs=wt2[:], start=True, stop=True)
                    nc.vector.tensor_scalar_mul(
                        out=agginit[:, i * C:(i + 1) * C], in0=p2[:],
                        scalar1=dinv[:, i:i + 1])
            itall = p3i.tile([128, F], dt.int32)
            nc.sync.dma_start(out=itall[:], in_=idxs.ap())
            col0 = 0
            for pieces, ccols in chunks:
                gb = p3g.tile([128, GMAX * C], dt.float32, tag="gb")
                for g in range(ccols):
                    nc.gpsimd.indirect_dma_start(
                        out=gb[:, g * C:(g + 1) * C],
                        out_offset=None,
                        in_=table.ap(),
                        in_offset=bass.IndirectOffsetOnAxis(
                            ap=itall[:, col0 + g:col0 + g + 1], axis=0))
                for (k, n, coff, soff) in pieces:
                    base = gb[:]
                    src = bass.AP(
                        base.tensor,
                        base.offset + coff * C,
                        [list(base.ap[0]), [k * C, n], [1, C], [C, k]])
                    nc.vector.tensor_reduce(
                        out=agg[:, soff * C:(soff + n) * C].rearrange(
                            "p (i c) -> p i c", c=C),
                        in_=src, axis=mybir.AxisListType.X,
                        op=mybir.AluOpType.add)
                col0 += ccols

            nc.vector.tensor_tensor(out=agg[:], in0=agg[:], in1=agginit[:],
                                    op=mybir.AluOpType.add)

            # ---------------- phase 4: loss -----------------------------
            with tc.tile_pool(name="p4", bufs=1) as p4, \
                 tc.tile_pool(name="ps4", bufs=1, space="PSUM") as ps4:
                yt = p4.tile([128, S * C], dt.float32)
                nc.sync.dma_start(out=yt[:], in_=y1h.ap())
                bt = p4.tile([128, C], dt.float32)
                nc.sync.dma_start(out=bt[:], in_=brep.ap())

                a3 = agg[:].rearrange("p (i c) -> p i c", c=C)
                nc.vector.tensor_tensor(
                    out=a3, in0=a3,
                    in1=dinv[:].rearrange("p (i one) -> p i one", one=1).to_broadcast(
                        [128, S, C]),
                    op=mybir.AluOpType.mult)
                nc.vector.tensor_tensor(
                    out=a3, in0=a3,
                    in1=bt[:].rearrange("p (one c) -> p one c", one=1).to_broadcast(
                        [128, S, C]),
                    op=mybir.AluOpType.add)
                mx = p4.tile([128, S], dt.float32)
                nc.vector.tensor_reduce(out=mx[:], in_=a3,
                                        axis=mybir.AxisListType.X,
                                        op=mybir.AluOpType.max)
                ex = p4.tile([128, S * C], dt.float32)
                e3 = ex[:].rearrange("p (i c) -> p i c", c=C)
                nc.vector.tensor_tensor(
                    out=e3, in0=a3,
                    in1=mx[:].rearrange("p (i one) -> p i one", one=1).to_broadcast(
                        [128, S, C]),
                    op=mybir.AluOpType.subtract)
                nc.scalar.activation(out=ex[:], in_=ex[:],
                                     func=mybir.ActivationFunctionType.Exp)
                se = p4.tile([128, S], dt.float32)
                nc.vector.tensor_reduce(out=se[:], in_=e3,
                                        axis=mybir.AxisListType.X,
                                        op=mybir.AluOpType.add)
                nc.scalar.activation(out=se[:], in_=se[:],
                                     func=mybir.ActivationFunctionType.Ln)
                lse = p4.tile([128, S], dt.float32)
                nc.vector.tensor_tensor(out=lse[:], in0=se[:], in1=mx[:],
                                        op=mybir.AluOpType.add)
                # mask = rowsum(y1h) computed BEFORE y3 is overwritten
                y3 = yt[:].rearrange("p (i c) -> p i c", c=C)
                msk = p4.tile([128, S], dt.float32)
                nc.vector.tensor_reduce(out=msk[:], in_=y3,
                                        axis=mybir.AxisListType.X,
                                        op=mybir.AluOpType.add)
                nc.vector.tensor_tensor(out=y3, in0=y3, in1=a3,
                                        op=mybir.AluOpType.mult)
                oy = p4.tile([128, S], dt.float32)
                nc.vector.tensor_reduce(out=oy[:], in_=y3,
                                        axis=mybir.AxisListType.X,
                                        op=mybir.AluOpType.add)
                term = p4.tile([128, S], dt.float32)
                nc.vector.tensor_tensor(out=term[:], in0=msk[:], in1=lse[:],
                                        op=mybir.AluOpType.mult)
                nc.vector.tensor_tensor(out=term[:], in0=term[:], in1=oy[:],
                                        op=mybir.AluOpType.subtract)
                part = p4.tile([128, 1], dt.float32)
                nc.vector.tensor_reduce(out=part[:], in_=term[:],
                                        axis=mybir.AxisListType.X,
                                        op=mybir.AluOpType.add)
                ones = p4.tile([128, 1], dt.float32)
                nc.gpsimd.memset(ones[:], 1.0)
                tot = ps4.tile([1, 1], dt.float32, space="PSUM")
                nc.tensor.matmul(out=tot[:], lhsT=part[:], rhs=ones[:],
                                 start=True, stop=True)
                res = p4.tile([1, 1], dt.float32)
                nc.vector.tensor_copy(out=res[:], in_=tot[:])
                nc.sync.dma_start(out=losss.ap(), in_=res[:])

    nc.compile()
    return nc


def _prep_inputs(hiddens, edge_index, y, W, b):
    src = np.asarray(edge_index[0], dtype=np.int64)
    dst = np.asarray(edge_index[1], dtype=np.int64)
    hid = np.asarray(hiddens, dtype=np.float32)
    yv = np.asarray(y, dtype=np.int64)

    # degree-balanced node->core assignment: deal degree-sorted nodes round
    # robin so all 8 per-bucket histograms match (shared-NEFF q = max over
    # cores).  newpos[v] = global table row of node v after the relabeling.
    deg_full = np.bincount(dst, minlength=N).astype(np.int64) + 1
    dorder = np.argsort(np.maximum(deg_full - 1, 1), kind="stable")
    node_core = np.empty(N, np.int64)
    node_core[dorder] = np.arange(N) % NCORES
    nodes_of = [np.where(node_core == r)[0] for r in range(NCORES)]
    newpos = np.empty(N, np.int64)
    for r in range(NCORES):
        newpos[nodes_of[r]] = r * NSH + np.arange(len(nodes_of[r]))

    shard = node_core[dst]
    order = np.argsort(shard, kind="stable")
    src_s, dst_s = src[order], dst[order]
    bounds = np.searchsorted(shard[order], np.arange(NCORES + 1))

    deg_by_core, csr = [], []
    for r in range(NCORES):
        s, e = bounds[r], bounds[r + 1]
        dl = newpos[dst_s[s:e]] - r * NSH
        sg = newpos[src_s[s:e]]           # table rows are newpos-ordered
        o2 = np.argsort(dl, kind="stable")
        dl, sg = dl[o2], sg[o2]
        deg = np.bincount(dl, minlength=NSH) + 1   # incl self loop
        csr.append((np.cumsum(np.bincount(dl, minlength=NSH)), sg))
        deg_by_core.append(deg)

    layout = _plan_layout(deg_by_core)
    buckets, chunks, S, F = layout

    in_maps = []
    for r in range(NCORES):
        deg = deg_by_core[r]
        endptr, sg = csr[r]
        startptr = np.concatenate([[0], endptr[:-1]])
        kidx = np.searchsorted(K_LIST, np.maximum(deg - 1, 1))

        idx_arr = np.full((128, F), ZROW, np.int32)
        degs_arr = np.ones((128, S), np.float32)
        y1h_arr = np.zeros((128, S, C), np.float32)
        slot_node = np.full(S * 128, -1, np.int64)   # (i*128+p) -> node

        col0 = 0
        slot0 = 0
        for bi, (k, qk) in enumerate(buckets):
            ki = K_LIST.index(k)
            nodes = np.where(kidx == ki)[0]
            for j, n in enumerate(nodes):
                p, i = j % 128, j // 128
                c0 = col0 + i * k
                nsrc = sg[startptr[n]:endptr[n]]
                idx_arr[p, c0:c0 + len(nsrc)] = nsrc
                degs_arr[p, slot0 + i] = deg[n]
                y1h_arr[p, slot0 + i, yv[nodes_of[r][n]]] = 1.0
                slot_node[(slot0 + i) * 128 + p] = n
            col0 += qk * k
            slot0 += qk

        hs = hid[nodes_of[r]]                               # [NSH, 128]
        # column order (i*QP + q) -> node q*QI + i
        ht = np.ascontiguousarray(
            hs.T.reshape(HID, QP, QI).transpose(0, 2, 1).reshape(HID, NSH))
        degn = np.ascontiguousarray(
            deg.astype(np.float32).reshape(QP, QI))

        ht2 = np.zeros((HID, S * 128), np.float32)
        valid = slot_node >= 0
        ht2[:, np.where(valid)[0]] = hs.T[:, slot_node[valid]]

        in_maps.append({
            "ht": ht,
            "ht2": ht2,
            "wmat": np.asarray(W, np.float32),
            "degn": degn,
            "idxs": idx_arr,
            "degs": degs_arr,
            "y1h": y1h_arr.reshape(128, S * C),
            "brep": np.broadcast_to(np.asarray(b, np.float32),
                                    (128, C)).copy(),
        })
    return in_maps, layout


def kernel(hiddens, edge_index, y, q_edge_index, W, b, trace=False):
    global _COMPILED
    from concourse import bass_utils

    in_maps, layout = _prep_inputs(hiddens, edge_index, y, W, b)
    if _COMPILED is None:
        _COMPILED = _build_program(layout)
    nc = _COMPILED
    res = bass_utils.run_bass_kernel_spmd(
        nc, in_maps, core_ids=list(range(NCORES)), trace=trace)
    total = sum(float(r["losss"][0, 0]) for r in res.results)
    out = np.float32(total / N)
    if trace:
        return out, res
    return out

